# revision 1
# baseline (speedup 1.0000x reference)
"""Mistral-style GQA attention block (B=1, S=2048, HID=4096, 32 q heads /
8 kv heads, head_dim=128, RoPE, causal) on 8 Trainium2 NeuronCores.

Sharding: tensor-parallel over heads. Core c owns q heads [4c, 4c+4) and
kv head c: Wq/Wk/Wv column-sharded, Wo row-sharded; the o_proj partial
products are summed on the host (the all-reduce of the TP scheme).

Device kernel layout notes (per core):
  All matmuls use natural operand layouts -- no on-device transposes of
  activations except V (16 small PE transposes):
    Q^T[d,m] = Wq_chunk.T @ X^T_chunk      (d on partitions)
    S^T[k,q] = (K^T chunk).T @ Q^T chunk   (softmax runs over partitions)
    den      = ones.T @ exp(S^T)           (PE reduction over partitions;
                                            ones is [128,128] so the
                                            denominator lands broadcast
                                            across all 128 partitions --
                                            no separate broadcast step)
    O^T[d,q] = V_chunk.T @ exp(S^T)
    Y[m,n]   = (O^T chunk).T @ Wo chunk
  Causality: only lower-triangle k-tiles are computed; diagonal tiles are
  fixed up by multiplying exp(S^T) with a 0/1 staircase mask on DVE (keeps
  the mask off PSUM and off the S->exp critical path).

  Projections and o_proj run in residual-split fp8 (DoubleRow perf mode,
  256-deep contraction at 0.5 PE cycles/row): X@W ~= Xhi@Whi + Xhi@Wlo +
  Xlo16@(Whi/16), with weights pre-scaled by 64 on the host and the 1/64
  folded into the PSUM-draining activation copies. Attention itself
  (RoPE'd Q/K, V, exp tiles) runs in bf16; per-core y partials ship as f16
  and are summed on the host.
"""

import os
import numpy as np
from contextlib import ExitStack

import ml_dtypes

import concourse.bass as bass
from concourse import bacc
import concourse.tile as tile
from concourse import mybir
from concourse.bass_utils import run_bass_kernel_spmd
from concourse.masks import make_identity

AF = mybir.ActivationFunctionType
DR = mybir.MatmulPerfMode.DoubleRow
F32 = mybir.dt.float32
F16 = mybir.dt.float16
BF16 = mybir.dt.bfloat16
FP8 = mybir.dt.float8e4
NP_BF16 = ml_dtypes.bfloat16
NP_FP8 = ml_dtypes.float8_e4m3

S = 2048          # sequence length
HID = 4096        # hidden size
D = 128           # head dim
NCORES = 8
HPC = 4           # q heads per core
DPC = HPC * D     # 512 q-proj columns per core
MC = 512          # seq chunk (free dim of most matmuls)
NKC = HID // 128  # 32 contraction chunks for projections
NPAIR = NKC // 2  # 16 DoubleRow contraction pairs for projections
NJC = S // MC     # 4 q chunks
NMT = S // 128    # 16 seq tiles of 128
SCALE = float(1.0 / np.sqrt(D))
WS = 64.0         # fp8 weight pre-scale (folded back in PSUM drains)
ROPE_THETA = 10000.0

# fp8 phase-A pair-block layout (bytes per partition row). The xkv stream
# carries only X (hi|lo16); the K/V projection weights are identical for
# every mc chunk, so they load once into wkv_sb instead of riding the
# stream 4x.
XHI, XLO = 0, 1024
PBLK = 2048
WKVP = 1024           # per-pair wkv block: khi|vhi|klo|vlo (hi/16 copies
                      # are derived on device: exact fp8 exponent shift)
# bf16 phase-A chunk layout
FB = MC + 2 * D       # 768: one fused xkv chunk (x | wk | wv)

LAST_RESULTS = None  # BassKernelResults of the most recent run (for test.py)


def _env(name, dflt):
    return int(os.environ.get(name, str(dflt)))


FP8A = bool(_env("KERNEL_FP8A", 1))
FP8C = bool(_env("KERNEL_FP8C", 1))


def _rope_drain(nc, pool, src_ps, tag, scale, bufs=2):
    """ACT copy draining the PSUM bank (frees it for the next accumulation
    group after one op)."""
    src = pool.tile([128, MC], BF16, tag=f"rsc{tag}", bufs=bufs,
                    name=f"ropesrc{tag}")
    nc.scalar.activation(src, src_ps, AF.Copy, scale=scale)
    return src


def _rope_dve(nc, pool, out, src, cos, sin, eng=None):
    """out = src*cos + rotate_half(src)*sin, all [128, MC] bf16 in SBUF:
    a 5-op elementwise chain (2x mode on DVE)."""
    lo = slice(0, 64)
    hi = slice(64, 128)
    eng = eng or nc.vector
    tmp = pool.tile([128, MC], BF16, tag="ropetmp", bufs=2, name="ropetmp")
    # sin is HALF-SWAPPED on the host (sin_sw[d] = sin[(d+64)%128]) so both
    # SBUF inputs of each mul share a base partition (walrus constraint).
    eng.tensor_mul(out, src, cos)
    eng.tensor_mul(tmp[lo, :], src[hi, :], sin[hi, :])
    eng.tensor_mul(tmp[hi, :], src[lo, :], sin[lo, :])
    eng.tensor_sub(out[lo, :], out[lo, :], tmp[lo, :])
    eng.tensor_add(out[hi, :], out[hi, :], tmp[hi, :])


def _pair3(ap1024):
    """[128, 1024] slice -> [128, 2, 512] DoubleRow operand view."""
    return ap1024.rearrange("p (j n) -> p j n", j=2)


def _emit(nc, xkv, wq, wkv, wo, cosT, sinT, maskT, y, tc):
    _ACCP = set(int(c) for c in os.environ.get('KERNEL_ACCPOOL', ''))
    NG = _env("KERNEL_XKV_GROUP", 4)
    XB = _env("KERNEL_XKV_BUFS", 2)
    SB = _env("KERNEL_S_BUFS", 2)
    OB = _env("KERNEL_O_BUFS", 3)
    DB = _env("KERNEL_DEN_BUFS", 1)
    YB = _env("KERNEL_Y_BUFS", 2)
    EXB = _env("KERNEL_EX_BUFS", 4)
    TRB = _env("KERNEL_TR_BUFS", 2)
    YRB = _env("KERNEL_YROW_BUFS", 6)
    nchunk = NPAIR if FP8A else NKC           # contraction steps per mc
    CW = PBLK if FP8A else FB                 # cols per contraction step
    GW = NG * CW                              # cols per DMA group
    assert nchunk % NG == 0, "xkv group size must divide the chunk count"
    ngrp = nchunk // NG
    drain_scale = (1.0 / WS) if FP8A else 1.0

    with ExitStack() as ctx:
        const = ctx.enter_context(tc.tile_pool(name="const", bufs=1))

        ident = const.tile([128, 128], BF16, name="ident")
        make_identity(nc, ident)
        ones_sq = const.tile([128, 128], BF16, name="ones_sq")
        nc.gpsimd.memset(ones_sq, 1.0)
        ones_f16 = const.tile([128, 128], F16, name="ones_f16")
        nc.gpsimd.memset(ones_f16, 1.0)
        nln16 = const.tile([128, 1], F32, name="nln16")
        nc.gpsimd.memset(nln16, -float(np.log(16.0)))

        cos_sb = const.tile([D, S], BF16, name="cos_sb")
        sin_sb = const.tile([D, S], BF16, name="sin_sb")
        # mask_sb[:, MC*t + b][a] = 0/1 keep-mask(q=b, k=128*t+a): the 4
        # staircase patterns for the k-tiles straddling the causal diagonal,
        # applied multiplicatively to exp(S^T).
        mask_sb = const.tile([128, 4 * MC], BF16, name="mask_sb")
        if FP8C:
            wo_sb = const.tile([128, 2 * 8 * 3 * 1024], FP8, name="wo_sb")
        else:
            wo_sb = const.tile([128, HPC * 8 * 512], BF16, name="wo_sb")

        qt = [const.tile([D, S], BF16, name=f"qt{h}") for h in range(HPC)]
        kt = const.tile([D, S], BF16, name="kt")
        vsb = const.tile([128, S], BF16, name="vsb")  # vsb[:, 128i:+128] = V rows 128i..
        if FP8C:
            othi = const.tile([128, HPC, S], FP8, name="othi")
            otlo = const.tile([128, HPC, S], FP8, name="otlo")
            ot = None
        else:
            ot = [const.tile([D, S], BF16, name=f"ot{h}") for h in range(HPC)]

        deferred_rope = []
        # ---------------- Phase A: projections + RoPE + V transpose -------
        with tc.tile_pool(name="pa", bufs=1) as pa, \
             tc.tile_pool(name="pap", bufs=1, space="PSUM") as pap:
            # wq/wkv live in the phase-A pool so their SBUF is recycled for
            # the B/C scratch tiles.
            if FP8A:
                wq_sb = pa.tile([128, NPAIR * 2048], FP8, name="wq_sb")
                wkv_sb = pa.tile([128, NPAIR * WKVP], FP8, name="wkv_sb")
                # device-derived hi/16 weight versions (term 3 of the
                # residual-split product)
                wq16_sb = pa.tile([128, NPAIR * 1024], FP8, name="wq16_sb")
                wkv16_sb = pa.tile([128, NPAIR * 512], FP8, name="wkv16_sb")
            else:
                wq_sb = pa.tile([128, NKC * MC], BF16, name="wq_sb")
                wkv_sb = None

            # Deferred const DMAs: interleaved with the first xkv group DMAs
            # so the projection stream never waits behind a bulk upfront
            # transfer. Weight chunk j must land before xkv group j is
            # consumed, hence chunking at group granularity.
            wq_cols = wq_sb.shape[1]
            NWQ = 4

            def wq_dma(j):
                def emit():
                    w = wq_cols // NWQ
                    if j == 0:
                        # quartered so the first projection matmul's weights
                        # land as early as possible
                        for q4 in range(4):
                            nc.scalar.dma_start(
                                wq_sb[:, w * q4 // 4:w * (q4 + 1) // 4],
                                wq[:, w * q4 // 4:w * (q4 + 1) // 4])
                    else:
                        nc.sync.dma_start(wq_sb[:, w * j:w * (j + 1)],
                                          wq[:, w * j:w * (j + 1)])
                return emit

            def wkv_dma(j):
                def emit():
                    w = wkv_sb.shape[1] // 4
                    if j == 0:
                        # split at pair granularity for the fastest start
                        for q4 in range(4):
                            nc.scalar.dma_start(
                                wkv_sb[:, w * q4 // 4:w * (q4 + 1) // 4],
                                wkv[:, w * q4 // 4:w * (q4 + 1) // 4])
                    else:
                        nc.sync.dma_start(wkv_sb[:, w * j:w * (j + 1)],
                                          wkv[:, w * j:w * (j + 1)])
                return emit

            def mask_dma(t):
                def emit():
                    nc.sync.dma_start(mask_sb[:, MC * t:MC * (t + 1)],
                                      maskT[128 * t:128 * (t + 1), :])
                return emit

            def wo_dma(j):
                def emit():
                    w = wo_sb.shape[1] // 8
                    nc.sync.dma_start(wo_sb[:, w * j:w * (j + 1)],
                                      wo[:, w * j:w * (j + 1)])
                return emit

            if FP8A:
                pending = [wq_dma(1), wkv_dma(1), wq_dma(2), wkv_dma(2),
                           wq_dma(3), wkv_dma(3),
                           lambda: nc.sync.dma_start(cos_sb, cosT),
                           lambda: nc.sync.dma_start(sin_sb, sinT)]
            else:
                pending = ([wq_dma(j) for j in range(1, NWQ)]
                           + [lambda: nc.sync.dma_start(cos_sb, cosT),
                              lambda: nc.sync.dma_start(sin_sb, sinT)])
            pending += [mask_dma(t) for t in range(4)] + \
                       [wo_dma(j) for j in range(8)]
            wq_dma(0)()
            if FP8A:
                wkv_dma(0)()
            for mc in range(NJC):
                ms = slice(MC * mc, MC * (mc + 1))
                ps_q = [pap.tile([128, MC], F32, tag=f"q{h}", name=f"ps_q{h}_{mc}")
                        for h in range(HPC)]
                ps_k = pap.tile([128, MC], F32, tag="k", name=f"ps_k_{mc}")
                ps_v = pap.tile([128, MC], F32, tag="v", name=f"ps_v_{mc}")
                for g in range(ngrp):
                    big = pa.tile([128, GW], FP8 if FP8A else BF16, tag="xkv",
                                  bufs=XB, name=f"xkv_{mc}_{g}")
                    gbase = GW * (ngrp * mc + g)
                    if mc == 0 and g == 0:
                        # split the very first transfer chunk-wise so the
                        # first matmuls start after ~1/NG of the group lands
                        for c2 in range(NG):
                            nc.sync.dma_start(
                                big[:, CW * c2:CW * (c2 + 1)],
                                xkv[:, gbase + CW * c2:gbase + CW * (c2 + 1)])
                    else:
                        nc.sync.dma_start(big, xkv[:, gbase:gbase + GW])
                    # wq consumption outpaces a 1-per-group drip early on;
                    # afterwards spread the remaining const DMAs across the
                    # whole xkv stream to keep bandwidth demand flat.
                    for _ in range(2 if mc == 0 and g < 4 else 1):
                        if pending:
                            pending.pop(0)()
                    for c2 in range(NG):
                        ck = NG * g + c2
                        base = CW * c2
                        st = ck == 0
                        sp = ck == nchunk - 1
                        if FP8A:
                            xh = _pair3(big[:, base + XHI:base + XHI + 1024])
                            xl = _pair3(big[:, base + XLO:base + XLO + 1024])
                            if mc == 0:
                                # derive the hi/16 copies once per pair, just
                                # ahead of their first use (DVE is idle here)
                                nc.vector.tensor_scalar_mul(
                                    wq16_sb[:, 1024 * ck:1024 * (ck + 1)],
                                    wq_sb[:, 2048 * ck:2048 * ck + 1024],
                                    1.0 / 16.0)
                                nc.vector.tensor_scalar_mul(
                                    wkv16_sb[:, 512 * ck:512 * (ck + 1)],
                                    wkv_sb[:, WKVP * ck:WKVP * ck + 512],
                                    1.0 / 16.0)
                            wb = WKVP * ck
                            w16 = 512 * ck
                            wk3 = [w.rearrange("p (j n) -> p j n", j=2) for w in
                                   (wkv_sb[:, wb:wb + 256],
                                    wkv_sb[:, wb + 512:wb + 768],
                                    wkv16_sb[:, w16:w16 + 256])]
                            wv3 = [w.rearrange("p (j n) -> p j n", j=2) for w in
                                   (wkv_sb[:, wb + 256:wb + 512],
                                    wkv_sb[:, wb + 768:wb + 1024],
                                    wkv16_sb[:, w16 + 256:w16 + 512])]
                            for h in range(HPC):
                                hsl = slice(128 * h, 128 * (h + 1))
                                wq3 = [_pair3(w)[:, :, hsl] for w in
                                       (wq_sb[:, 2048 * ck:2048 * ck + 1024],
                                        wq_sb[:, 2048 * ck + 1024:
                                              2048 * (ck + 1)],
                                        wq16_sb[:, 1024 * ck:1024 * (ck + 1)])]
                                nc.tensor.matmul(ps_q[h], wq3[0], xh,
                                                 start=st, stop=False,
                                                 perf_mode=DR)
                                nc.tensor.matmul(ps_q[h], wq3[1], xh,
                                                 start=False, stop=False,
                                                 perf_mode=DR)
                                nc.tensor.matmul(ps_q[h], wq3[2], xl,
                                                 start=False, stop=sp,
                                                 perf_mode=DR)
                            for ps, w3 in ((ps_k, wk3), (ps_v, wv3)):
                                nc.tensor.matmul(ps, w3[0], xh, start=st,
                                                 stop=False, perf_mode=DR)
                                nc.tensor.matmul(ps, w3[1], xh, start=False,
                                                 stop=False, perf_mode=DR)
                                nc.tensor.matmul(ps, w3[2], xl, start=False,
                                                 stop=sp, perf_mode=DR)
                        else:
                            xt_ = big[:, base:base + MC]
                            wk_ = big[:, base + MC:base + MC + D]
                            wv_ = big[:, base + MC + D:base + FB]
                            wqc = wq_sb[:, MC * ck:MC * (ck + 1)]
                            for h in range(HPC):
                                nc.tensor.matmul(ps_q[h],
                                                 wqc[:, D * h:D * (h + 1)],
                                                 xt_, start=st, stop=sp)
                            nc.tensor.matmul(ps_k, wk_, xt_, start=st, stop=sp)
                            nc.tensor.matmul(ps_v, wv_, xt_, start=st, stop=sp)
                # V path first: the PE transposes only wait on the ACT copy,
                # never on the DVE rope backlog.
                vt_ = pa.tile([128, MC], BF16, tag="vt", bufs=2, name=f"vt_{mc}")
                nc.scalar.activation(vt_, ps_v, AF.Copy, scale=drain_scale)
                for b in range(4):
                    ps_t = pap.tile([128, 128], BF16, tag="tr", bufs=TRB,
                                    name=f"ps_tr_{mc}_{b}")
                    nc.tensor.transpose(ps_t, vt_[:, 128 * b:128 * (b + 1)], ident)
                    i = 4 * mc + b
                    nc.vector.tensor_copy(vsb[:, 128 * i:128 * (i + 1)], ps_t)
                # The last chunk's 25-op DVE rope tail would otherwise
                # outrank B(0)'s softmax chain in the DVE queue right at the
                # A->B boundary: drain its PSUM banks here (ACT), but defer
                # the DVE math until after B(0) is emitted. B(3), its only
                # consumer, runs much later.
                last = mc == NJC - 1
                pool = const if last else pa
                for h in range(HPC):
                    srcq = _rope_drain(nc, pool, ps_q[h],
                                       f"{h}_{mc}" if last else h,
                                       drain_scale, bufs=1 if last else 2)
                    if last:
                        deferred_rope.append((qt[h], srcq, ms))
                    else:
                        _rope_dve(nc, pa, qt[h][:, ms], srcq, cos_sb[:, ms],
                                  sin_sb[:, ms])
                srck = _rope_drain(nc, pool, ps_k, f"k_{mc}" if last else 'k',
                                   drain_scale, bufs=1 if last else 2)
                if last:
                    deferred_rope.append((kt, srck, ms))
                else:
                    _rope_dve(nc, pa, kt[:, ms], srck, cos_sb[:, ms],
                              sin_sb[:, ms])

        # ---------------- Phases B+C interleaved --------------------------
        phases = os.environ.get("KERNEL_PHASES", "ABC")
        if "B" not in phases:
            return
        with tc.tile_pool(name="pb", bufs=1) as pb, \
             tc.tile_pool(name="pbp", bufs=1, space="PSUM") as pbp:

            def B(jc):
                qs = slice(MC * jc, MC * (jc + 1))
                nk = 4 * jc + 4
                # For the early q-chunks the exp tiles are pre-summed on DVE
                # (bf16 2x adds) so the PE only runs ONE partition-reducing
                # den matmul per (h, jc). The last chunk keeps the den
                # accumulation on PE: there the DVE chain latency would sit
                # on the kernel's tail.
                dve_den = jc < _env('KERNEL_DVEDEN', 4)
                # for the last chunk, close the denominator on PE (final 4
                # tiles) so the reciprocal never waits on the DVE add chain,
                # which is saturated in the kernel tail
                mix_tail = (jc == 3 and dve_den
                            and _env('KERNEL_DENMIX', 0)) and 4 or 0
                for h in range(HPC):
                    ps_o = pbp.tile([128, MC], F32, tag="o", bufs=OB,
                                    name=f"ps_o_{h}_{jc}")
                    # dve_den's single den matmul shares the ps_y rotation
                    # (same shape/pool) so ps_s can take the freed bank
                    ps_den = pbp.tile([128, MC], F32,
                                      tag="y" if dve_den and _env("KERNEL_DEN_SHARE", 0) else "den",
                                      bufs=YB if dve_den and _env("KERNEL_DEN_SHARE", 0) else DB,
                                      name=f"ps_den_{h}_{jc}")
                    acc = None
                    for i in range(nk):
                        ks = slice(128 * i, 128 * (i + 1))
                        ps_s = pbp.tile([128, MC], F32, tag="s", bufs=SB,
                                        name=f"ps_s_{h}_{jc}_{i}")
                        tt = i - 4 * jc
                        nc.tensor.matmul(ps_s, kt[:, ks], qt[h][:, qs],
                                         start=True, stop=True)
                        ex = pb.tile([128, MC], BF16, tag="ex", bufs=EXB,
                                     name=f"ex_{h}_{jc}_{i}")
                        # exp(s*SCALE - ln16) = exp(s*SCALE)/16: keeps the
                        # f16 denominator accumulator 16x below its range
                        # limit; the softmax normalization cancels the factor
                        # exactly (O and den scale together).
                        nc.scalar.activation(ex, ps_s, AF.Exp, scale=SCALE,
                                             bias=nln16)
                        if tt >= 0:
                            nc.vector.tensor_mul(ex, ex,
                                                 mask_sb[:, MC * tt:MC * (tt + 1)])
                        st = i == 0
                        sp = i == nk - 1
                        nc.tensor.matmul(ps_o, vsb[:, ks], ex, start=st, stop=sp)
                        if dve_den and i >= nk - mix_tail:
                            if i == nk - mix_tail:
                                nc.tensor.matmul(ps_den, ones_f16, acc,
                                                 start=True, stop=False)
                            nc.tensor.matmul(ps_den, ones_sq, ex,
                                             start=False, stop=sp)
                        elif dve_den:
                            aeng = (nc.gpsimd if jc in _ACCP else nc.vector)
                            if acc is None:
                                # f16: 2-byte (keeps DVE 2x mode) with enough
                                # mantissa that the running sum stays accurate;
                                # denominators peak ~30k, under f16 max 65504
                                acc = pb.tile([128, MC], F16, tag="exacc",
                                              bufs=2, name=f"exacc_{h}_{jc}")
                                aeng.tensor_copy(acc, ex)
                            else:
                                aeng.tensor_add(acc, acc, ex)
                        else:
                            nc.tensor.matmul(ps_den, ones_sq, ex, start=st,
                                             stop=sp)
                    if dve_den and not mix_tail:
                        nc.tensor.matmul(ps_den, ones_f16, acc,
                                         start=True, stop=True)
                    if FP8C:
                        recip = pb.tile([128, MC], F32, tag="recip", bufs=2,
                                        name=f"recip_{h}_{jc}")
                        of32 = pb.tile([128, MC], F32, tag="of32",
                                       bufs=_env("KERNEL_OF_BUFS", 2),
                                       name=f"of32_{h}_{jc}")
                        ores = pb.tile([128, MC], F32, tag="ores",
                                       bufs=_env("KERNEL_OF_BUFS", 2),
                                       name=f"ores_{h}_{jc}")
                        # the last chunk's chain gates phase C on the kernel
                        # tail: pipeline it in column halves there
                        nhalf = 2 if jc == 3 and _env("KERNEL_OT_HALVES", 1)                             else 1
                        hw_ = MC // nhalf
                        for hh in range(nhalf):
                            sl = slice(hw_ * hh, hw_ * (hh + 1))
                            qsl = slice(MC * jc + hw_ * hh,
                                        MC * jc + hw_ * (hh + 1))
                            nc.vector.reciprocal(recip[:, sl], ps_den[:, sl])
                            nc.vector.tensor_mul(of32[:, sl], ps_o[:, sl],
                                                 recip[:, sl])
                            nc.scalar.activation(othi[:, h, qsl], of32[:, sl],
                                                 AF.Copy)
                            nc.vector.tensor_sub(ores[:, sl], of32[:, sl],
                                                 othi[:, h, qsl])
                            nc.scalar.activation(otlo[:, h, qsl], ores[:, sl],
                                                 AF.Copy, scale=16.0)
                    else:
                        recip = pb.tile([128, MC], F32, tag="recip", bufs=2,
                                        name=f"recip_{h}_{jc}")
                        nc.vector.reciprocal(recip, ps_den)
                        nc.vector.tensor_mul(ot[h][:, qs], ps_o, recip)

            def C(j):
                for mt in range(4 * j, 4 * j + 4):
                    mts = slice(128 * mt, 128 * (mt + 1))
                    yrow = pb.tile([128, HID], F16, tag="yrow", bufs=YRB,
                                   name=f"yrow_{mt}")
                    last = mt == NMT - 1
                    for nb in range(8):
                        ps_y = pbp.tile([128, 512], F32, tag="y", bufs=YB,
                                        name=f"ps_y_{mt}_{nb}")
                        if FP8C:
                            for c in range(2):
                                l_hi = othi[:, 2 * c:2 * c + 2, mts]
                                l_lo = otlo[:, 2 * c:2 * c + 2, mts]
                                woff = ((c * 8 + nb) * 3) * 1024
                                w3 = [_pair3(wo_sb[:, woff + 1024 * v:
                                                   woff + 1024 * (v + 1)])
                                      for v in range(3)]
                                nc.tensor.matmul(ps_y, l_hi, w3[0],
                                                 start=(c == 0), stop=False,
                                                 perf_mode=DR)
                                nc.tensor.matmul(ps_y, l_hi, w3[1],
                                                 start=False, stop=False,
                                                 perf_mode=DR)
                                nc.tensor.matmul(ps_y, l_lo, w3[2],
                                                 start=False, stop=(c == 1),
                                                 perf_mode=DR)
                        else:
                            for dc in range(HPC):
                                wslice = wo_sb[:, 512 * (8 * dc + nb):
                                               512 * (8 * dc + nb + 1)]
                                nc.tensor.matmul(ps_y, ot[dc][:, mts], wslice,
                                                 start=(dc == 0),
                                                 stop=(dc == HPC - 1))
                        ys = slice(512 * nb, 512 * (nb + 1))
                        # split the PSUM drain across ACT and DVE
                        if nb % 2 == 0:
                            nc.scalar.activation(yrow[:, ys], ps_y, AF.Copy,
                                                 scale=(1.0 / WS) if FP8C
                                                 else 1.0)
                        elif FP8C:
                            nc.vector.tensor_scalar_mul(yrow[:, ys], ps_y,
                                                        1.0 / WS)
                        else:
                            nc.vector.tensor_copy(yrow[:, ys], ps_y)
                        if last:
                            # drip the final row out as it completes so the
                            # kernel tail isn't one long DMA
                            nc.sync.dma_start(y[mts, ys], yrow[:, ys])
                    if not last:
                        nc.sync.dma_start(y[mts, :], yrow)

            B(0)
            if _env("KERNEL_ROPE_AFTER", 0) == 0:
                deng = nc.gpsimd if _env("KERNEL_ROPE_POOL", 1) else None
                for out_t, src_t, ms_ in deferred_rope:
                    _rope_dve(nc, pb, out_t[:, ms_], src_t, cos_sb[:, ms_],
                              sin_sb[:, ms_], eng=deng)
            B(1)
            if _env("KERNEL_ROPE_AFTER", 0) == 1:
                for out_t, src_t, ms_ in deferred_rope:
                    _rope_dve(nc, pb, out_t[:, ms_], src_t, cos_sb[:, ms_],
                              sin_sb[:, ms_])
            corder = _env("KERNEL_CORDER", 0)
            if corder == 0:
                if "C" in phases:
                    C(0)
                B(2)
                if "C" in phases:
                    C(1)
                B(3)
                if "C" in phases:
                    C(2)
                    C(3)
            else:
                B(2)
                if "C" in phases:
                    C(0)
                B(3)
                if "C" in phases:
                    C(1)
                    C(2)
                    C(3)


_BUILT = None


def _build():
    global _BUILT
    if _BUILT is not None:
        return _BUILT
    nc = bacc.Bacc("TRN2", target_bir_lowering=False, debug=False,
                   num_devices=NCORES)
    if FP8A:
        xkv = nc.dram_tensor("xkv", [128, NJC * NPAIR * PBLK], FP8,
                             kind="ExternalInput").ap()
        wq = nc.dram_tensor("wq", [128, NPAIR * 2048], FP8,
                            kind="ExternalInput").ap()
        wkv = nc.dram_tensor("wkv", [128, NPAIR * WKVP], FP8,
                             kind="ExternalInput").ap()
    else:
        xkv = nc.dram_tensor("xkv", [128, NJC * NKC * FB], BF16,
                             kind="ExternalInput").ap()
        wq = nc.dram_tensor("wq", [128, NKC * MC], BF16,
                            kind="ExternalInput").ap()
        wkv = None
    if FP8C:
        wo = nc.dram_tensor("wo", [128, 2 * 8 * 3 * 1024], FP8,
                            kind="ExternalInput").ap()
    else:
        wo = nc.dram_tensor("wo", [128, HPC * 8 * 512], BF16,
                            kind="ExternalInput").ap()
    cosT = nc.dram_tensor("cosT", [D, S], BF16, kind="ExternalInput").ap()
    sinT = nc.dram_tensor("sinT", [D, S], BF16, kind="ExternalInput").ap()
    maskT = nc.dram_tensor("maskT", [MC, MC], BF16, kind="ExternalInput").ap()
    y = nc.dram_tensor("y", [S, HID], F16, kind="ExternalOutput").ap()
    with tile.TileContext(nc) as tc:
        _emit(nc, xkv, wq, wkv, wo, cosT, sinT, maskT, y, tc)
    nc.compile()
    _BUILT = nc
    return nc


def _fp8_split(m64):
    """m64: [rows, cols] f32 (already weight-scaled). Returns hi, lo, hi/16
    as fp8 arrays."""
    hi = m64.astype(NP_FP8)
    hif = hi.astype(np.float32)
    lo = (m64 - hif).astype(NP_FP8)
    hi16 = (hif / 16.0).astype(NP_FP8)
    return hi, lo, hi16


def _pairify(m):
    """[4096, W] -> [NPAIR, 128, 2, W]: [p, part, j, n] = m[256p+128j+part, n]."""
    return m.reshape(NPAIR, 2, 128, -1).transpose(0, 2, 1, 3)


def prep_in_maps(hidden_states, Wq, Wk, Wv, Wo, attention_mask, position_ids):
    hidden_states = np.asarray(hidden_states, dtype=np.float32)
    Wq = np.asarray(Wq, dtype=np.float32)
    Wk = np.asarray(Wk, dtype=np.float32)
    Wv = np.asarray(Wv, dtype=np.float32)
    Wo = np.asarray(Wo, dtype=np.float32)
    attention_mask = np.asarray(attention_mask, dtype=np.float32)
    position_ids = np.asarray(position_ids)

    xT = np.ascontiguousarray(hidden_states[0].T)  # [HID, S] f32

    # RoPE tables (host-precomputed from position_ids, as in the reference)
    pos = position_ids[0].astype(np.float32)  # [S]
    inv_freq = (1.0 / (ROPE_THETA ** (np.arange(0, D, 2, dtype=np.float32) / D))
                ).astype(np.float32)
    freqs = pos[:, None] * inv_freq[None, :]           # [S, D/2]
    emb = np.concatenate([freqs, freqs], axis=-1)      # [S, D]
    cosT = np.ascontiguousarray(np.cos(emb).T).astype(NP_BF16)  # [D, S]
    sinT = np.sin(emb).T.astype(np.float32)
    sinT = np.ascontiguousarray(
        np.concatenate([sinT[64:], sinT[:64]], axis=0)).astype(NP_BF16)

    # diagonal staircase keep-mask (1 = keep, 0 = masked), sliced from the
    # provided additive mask
    maskT = np.ascontiguousarray(
        (attention_mask[0, 0, :MC, :MC].T == 0.0)).astype(NP_BF16)  # [k, q]

    if FP8A:
        xhi = xT.astype(NP_FP8)
        xlo16 = ((xT - xhi.astype(np.float32)) * 16.0).astype(NP_FP8)
        xhi_p = _pairify(xhi)      # [p, part, j, S]
        xlo_p = _pairify(xlo16)
    else:
        xTr = xT.astype(NP_BF16).reshape(NKC, 128, S)

    in_maps = []
    for c in range(NCORES):
        if FP8A:
            # xkv pair blocks: [mc, pair, part, PBLK] (x hi|lo only)
            blocks = np.zeros((NJC, NPAIR, 128, PBLK), dtype=NP_FP8)
            for mc in range(NJC):
                msl = slice(MC * mc, MC * (mc + 1))
                blocks[mc, :, :, XHI:XHI + 1024] = (
                    xhi_p[:, :, :, msl].reshape(NPAIR, 128, 1024))
                blocks[mc, :, :, XLO:XLO + 1024] = (
                    xlo_p[:, :, :, msl].reshape(NPAIR, 128, 1024))
            xkv_c = blocks.transpose(2, 0, 1, 3).reshape(128, -1)
            # load-once K/V projection weights: [pair, part, khi|vhi|klo|vlo]
            # (hi/16 versions are derived on device)
            wkv_c = np.zeros((NPAIR, 128, WKVP), dtype=NP_FP8)
            for base, W in ((0, Wk), (256, Wv)):
                w3 = _fp8_split(WS * W[:, D * c:D * (c + 1)])
                for v in range(2):
                    wp = _pairify(w3[v].astype(np.float32)).astype(NP_FP8)
                    wkv_c[:, :, base + 512 * v:base + 512 * v + 256] = (
                        wp.reshape(NPAIR, 128, 256))
            wkv_c = wkv_c.transpose(1, 0, 2).reshape(128, -1)
            # wq: [pair, part, {hi,lo}, j, 512]
            wq3 = _fp8_split(WS * Wq[:, DPC * c:DPC * (c + 1)])
            wq_c = np.stack(
                [_pairify(w.astype(np.float32)).astype(NP_FP8)
                 for w in wq3[:2]],
                axis=2)  # [p, part, v, j, 512]
            wq_c = wq_c.transpose(1, 0, 2, 3, 4).reshape(128, -1)
        else:
            wk_c = Wk[:, D * c:D * (c + 1)].astype(NP_BF16).reshape(NKC, 128, D)
            wv_c = Wv[:, D * c:D * (c + 1)].astype(NP_BF16).reshape(NKC, 128, D)
            blocks = np.empty((NJC, NKC, 128, FB), dtype=NP_BF16)
            for mc in range(NJC):
                blocks[mc, :, :, :MC] = xTr[:, :, MC * mc:MC * (mc + 1)]
                blocks[mc, :, :, MC:MC + D] = wk_c
                blocks[mc, :, :, MC + D:] = wv_c
            xkv_c = blocks.transpose(2, 0, 1, 3).reshape(128, -1)
            wq_c = (Wq[:, DPC * c:DPC * (c + 1)].astype(NP_BF16)
                    .reshape(NKC, 128, DPC).transpose(1, 0, 2).reshape(128, -1))

        Wo_c = Wo[DPC * c:DPC * (c + 1), :]  # [512, 4096]
        if FP8C:
            wo3 = _fp8_split(WS * Wo_c)  # each [512, 4096]
            # [part, c, nb, v, j, 512]
            wo_c = np.zeros((128, 2, 8, 3, 2, 512), dtype=NP_FP8)
            for v in range(3):
                wp = (wo3[v].astype(np.float32)
                      .reshape(2, 2, 128, 8, 512))  # [c, j, part, nb, n]
                wo_c[:, :, :, v] = wp.transpose(2, 0, 3, 1, 4).astype(NP_FP8)
            wo_c = wo_c.reshape(128, -1)
        else:
            wo_c = (Wo_c.astype(NP_BF16)
                    .reshape(HPC, 128, 8, 512).transpose(1, 0, 2, 3)
                    .reshape(128, -1))
        im = {
            "xkv": np.ascontiguousarray(xkv_c),
            "wq": np.ascontiguousarray(wq_c),
            "wo": np.ascontiguousarray(wo_c),
            "cosT": cosT,
            "sinT": sinT,
            "maskT": maskT,
        }
        if FP8A:
            im["wkv"] = np.ascontiguousarray(wkv_c)
        in_maps.append(im)

    return in_maps


def kernel(hidden_states, Wq, Wk, Wv, Wo, attention_mask, position_ids):
    global LAST_RESULTS
    in_maps = prep_in_maps(hidden_states, Wq, Wk, Wv, Wo, attention_mask,
                           position_ids)
    nc = _build()
    res = run_bass_kernel_spmd(nc, in_maps, list(range(NCORES)),
                               trace=bool(int(os.environ.get("KERNEL_TRACE", "0"))))
    LAST_RESULTS = res

    acc = np.zeros((S, HID), dtype=np.float64)
    for c in range(NCORES):
        acc += res.results[c]["y"].astype(np.float64)
    return acc.astype(np.float32)[None]  # [1, S, HID]

